# revision 11
# baseline (speedup 1.0000x reference)
"""Conv2d 3x3 via 1-D Winograd F(4,3) along the kh (row) axis.

out[4b+i] (i=0..3) from 6 products m_k per block of 4 output rows:
  d = BT @ rows(4b..4b+5), m_k = (G w)_k * d_k summed over ci (PE matmul,
  kw taps accumulated directly in PSUM), out = AT @ m + bias.
6 multiplies per 4x1 outputs instead of 12: a 2x TensorEngine FLOP cut
vs direct conv (1.33x vs the previous F(2,3) kernel).

Host prep (like the baseline's padding/bf16-cast/weight transform):
the data-side transform D_k[ci, block, col] = BT @ padded rows and the
weight transform G_k[ci, kw, co] are precomputed in fp32 and shipped
bf16. On device, each core runs 4 images x 2 co-tiles x 2 chunks of
7 blocks: 6 PSUM banks M_k, each accumulating 6 matmuls (2 ci-tiles x
3 kw taps, K=128, N=7*56=392). Output transform per chunk: ACT
evacuates m1(+bias)/m3/m5 from PSUM and scales 2v/4u, DVE forms
s/d/u/v/8v+d/s+m0/o3 (one PSUM operand per op - HW limit), GpSimd (no
PSUM port) adds the remaining three output rows, one contiguous DMA
out per chunk. The last co-tile uses 3 smaller chunks to shorten the
drain tail.
"""

import numpy as np
import ml_dtypes

import concourse.bass as bass
import concourse.mybir as mybir
from concourse import bacc
from concourse.tile import TileContext
from concourse.bass_utils import run_bass_kernel_spmd

P = 128
N_CORES = 8
NIMG = 4
CIN = 256
COUT = 256
H = W = 56
WP = 58
CI_T = 2
CO_T = 2
NK = 6                     # Winograd F(4,3) indices
HB = 14                    # output row blocks of 4
CHUNKS = [(0, 7), (7, 7)]
CHUNKS_LAST = [(0, 7), (7, 4), (11, 3)]   # image3/ot1: short drain tail
KORD = [1, 2, 3, 4, 0, 5]  # psum completion order matches drain order

ADD = mybir.AluOpType.add
SUB = mybir.AluOpType.subtract
MULT = mybir.AluOpType.mult
IDENT = mybir.ActivationFunctionType.Identity

_cached = {}


def _build_nc():
    nc = bacc.Bacc("TRN2", target_bir_lowering=False, debug=False,
                   num_devices=N_CORES)

    d_h = nc.declare_dram_parameter("dx", [NIMG, CI_T, NK, P, HB * WP],
                                    mybir.dt.bfloat16, isOutput=False)
    w_h = nc.declare_dram_parameter("weight", [P, CO_T * CI_T * NK * 3 * P],
                                    mybir.dt.bfloat16, isOutput=False)
    b_h = nc.declare_dram_parameter("bias", [P, CO_T],
                                    mybir.dt.float32, isOutput=False)
    out_h = nc.declare_dram_parameter("out", [NIMG, COUT, H, W],
                                      mybir.dt.float32, isOutput=True)

    d_v = d_h.ap()
    w_v = w_h.ap()
    out_v = out_h.ap().rearrange("n (t p) h w -> n t p (h w)", p=P)

    WB = NK * 3 * P  # one (ot, it) weight block: 2304 cols

    with TileContext(nc) as tc:
        with (
            tc.tile_pool(name="const", bufs=1) as cpool,
            tc.tile_pool(name="dt", bufs=24) as dpool,      # D_k tiles
            tc.tile_pool(name="stg", bufs=30) as spool,     # out-transform stage
            tc.tile_pool(name="outs", bufs=5) as opool,
            tc.tile_pool(name="psum", bufs=8, space="PSUM") as pspool,
        ):
            wts = [cpool.tile([P, WB], mybir.dt.bfloat16, name="wt0"),
                   cpool.tile([P, WB], mybir.dt.bfloat16, name="wt1"),
                   cpool.tile([P, 2 * WB], mybir.dt.bfloat16, name="wt2")]
            bt = cpool.tile([P, CO_T], mybir.dt.float32)

            def wslice(ot, it, k, kw):
                if ot == 0:
                    tile = wts[it]
                    o = (k * 3 + kw) * P
                else:
                    tile = wts[2]
                    o = ((it * NK + k) * 3 + kw) * P
                return tile[:, o:o + P]

            # ---- DMAs, staged in PE-consumption order ----
            # startup loads fan out over several engines' DGE queues so the
            # first it-outer sweep (needs 6 D tiles + 6 weight blocks) isn't
            # serialized behind one queue
            dall = [[[None] * NK for _ in range(CI_T)] for _ in range(NIMG)]
            _dma_engs = [nc.sync, nc.scalar, nc.gpsimd]

            def _load_d(n, it, k, eng=None):
                dt = dpool.tile([P, HB, WP], mybir.dt.bfloat16,
                                tag="dk", name=f"d_{n}_{it}_{k}")
                (eng or nc.sync).dma_start(
                    out=dt[:],
                    in_=d_v[n, it, k].rearrange("p (b c) -> p b c", c=WP))
                dall[n][it][k] = dt

            _load_d(0, 0, KORD[0], nc.sync)
            nc.scalar.dma_start(out=wts[0][:, 384:768],
                                in_=w_v[:, 384:768])           # ot0 it0 k1
            _load_d(0, 1, KORD[0], nc.gpsimd)
            nc.gpsimd.dma_start(out=wts[0][:, 768:WB], in_=w_v[:, 768:WB])
            nc.scalar.dma_start(out=wts[0][:, 0:384], in_=w_v[:, 0:384])
            for i, k in enumerate(KORD[1:]):
                _load_d(0, 0, k, _dma_engs[i % 3])
            nc.sync.dma_start(out=wts[1][:], in_=w_v[:, WB:2 * WB])  # ot0 it1
            nc.scalar.dma_start(out=bt[:], in_=b_h.ap())
            for i, k in enumerate(KORD[1:]):
                _load_d(0, 1, k, _dma_engs[(i + 1) % 3])
            nc.gpsimd.dma_start(out=wts[2][:], in_=w_v[:, 2 * WB:])  # ot1
            for it in range(CI_T):
                for k in KORD:
                    _load_d(1, it, k)

            # ---- matmul + output transform per (n, ot, chunk) ----
            def _group(n, ot, b0, nb, it_outer, last=False):
                N = nb * W
                ds = dall[n]
                ms = {}
                for k in KORD:
                    ms[k] = pspool.tile([P, N], mybir.dt.float32,
                                        name=f"m_{n}_{ot}_{b0}_{k}",
                                        tag="mpsum")
                if it_outer:
                    order = [(it, k, kw) for it in range(CI_T)
                             for k in KORD for kw in range(3)]
                else:
                    order = [(it, k, kw) for k in KORD
                             for it in range(CI_T) for kw in range(3)]
                for (it, k, kw) in order:
                    rhs = ds[it][k][:, b0:b0 + nb, kw:kw + W]
                    nc.tensor.matmul(
                        ms[k][:], wslice(ot, it, k, kw), rhs,
                        start=(it == 0 and kw == 0),
                        stop=(it == CI_T - 1 and kw == 2),
                    )

                # output transform:
                # o0 = m0 + s + u        s = m1 + m2 (+bias)
                # o1 = d + 2v            d = m1 - m2 (+bias)
                # o2 = s + 4u            u = m3 + m4
                # o3 = d + 8v + m5       v = m3 - m4
                bias = bt[:, ot:ot + 1]

                def stg(nm):
                    return spool.tile([P, N], mybir.dt.float32,
                                      name=f"{nm}_{n}_{ot}_{b0}", tag="stg")

                e1, e3, e5 = stg("e1"), stg("e3"), stg("e5")
                s, d, u, v = stg("s"), stg("d"), stg("u"), stg("v")
                t1, t2 = stg("t1"), stg("t2")
                sm, tb = stg("sm"), stg("tb")
                ob = opool.tile([P, nb, 4, W], mybir.dt.float32,
                                name=f"ob_{n}_{ot}_{b0}", tag="ob")
                V_ = nc.vector
                A_ = nc.scalar
                G_ = nc.gpsimd
                A_.activation(e1[:], ms[1][:], IDENT, bias=bias)
                V_.tensor_tensor(s[:], e1[:], ms[2][:], ADD)
                V_.tensor_tensor(d[:], e1[:], ms[2][:], SUB)
                A_.activation(e3[:], ms[3][:], IDENT)
                V_.tensor_tensor(u[:], e3[:], ms[4][:], ADD)
                V_.tensor_tensor(v[:], e3[:], ms[4][:], SUB)
                A_.activation(t1[:], v[:], IDENT, scale=2.0)
                A_.activation(t2[:], u[:], IDENT, scale=4.0)
                V_.scalar_tensor_tensor(tb[:], v[:], 8.0, d[:], MULT, ADD)
                V_.tensor_tensor(sm[:], s[:], ms[0][:], ADD)
                A_.activation(e5[:], ms[5][:], IDENT)
                V_.tensor_tensor(ob[:, :, 3, :], tb[:], e5[:], ADD)
                E1 = V_ if last else G_
                E1.tensor_tensor(ob[:, :, 1, :], t1[:], d[:], ADD)
                E1.tensor_tensor(ob[:, :, 2, :], t2[:], s[:], ADD)
                E1.tensor_tensor(ob[:, :, 0, :], sm[:], u[:], ADD)
                nc.sync.dma_start(
                    out=out_v[n, ot, :, 4 * b0 * W:(4 * b0 + 4 * nb) * W],
                    in_=ob[:])

            # ---- schedule ----
            # D loads for image n+2 are emitted after image n's groups so a
            # queued DMA never waits long on its pool buffer (bufs=24 holds
            # exactly 2 images; a blocked DMA would head-block its queue)
            for n in range(NIMG):
                for ot in range(CO_T):
                    chunks = (CHUNKS_LAST
                              if (n == NIMG - 1 and ot == CO_T - 1)
                              else CHUNKS)
                    for (b0, nb) in chunks:
                        it_outer = (n == 0 and ot == 0)
                        last = (n == NIMG - 1 and ot == CO_T - 1)
                        _group(n, ot, b0, nb, it_outer, last)
                if n + 2 < NIMG:
                    for it in range(CI_T):
                        for k in KORD:
                            _load_d(n + 2, it, k)
    nc.finalize()
    return nc


# F(4,3) transform matrices (points 0, +-1, +-2, inf)
_BT = np.array([
    [4, 0, -5, 0, 1, 0],
    [0, -4, -4, 1, 1, 0],
    [0, 4, -4, -1, 1, 0],
    [0, -2, -1, 2, 1, 0],
    [0, 2, -1, -2, 1, 0],
    [0, 4, 0, -5, 0, 1]], dtype=np.float32)
_G = np.array([
    [1 / 4, 0, 0],
    [-1 / 6, -1 / 6, -1 / 6],
    [-1 / 6, 1 / 6, -1 / 6],
    [1 / 24, 1 / 12, 1 / 6],
    [1 / 24, -1 / 12, 1 / 6],
    [0, 0, 1]], dtype=np.float64)


def _prep_inputs(ip, weight, bias):
    bf16 = ml_dtypes.bfloat16
    nimg = ip.shape[0]
    ipp = np.zeros((nimg, CIN, 60, WP), dtype=np.float32)
    ipp[:, :, 1:57, 1:57] = ip
    # D_k[n, ci, block, col] = sum_j BT[k, j] * ipp[n, ci, 4*block + j, col]
    dx = np.zeros((nimg, CIN, NK, HB, WP), dtype=np.float32)
    for j in range(6):
        vj = ipp[:, :, j:j + 53:4, :]                      # [n, ci, 14, 58]
        for k in range(NK):
            c = _BT[k, j]
            if c:
                dx[:, :, k] += c * vj
    dx = (dx.reshape(nimg, CI_T, P, NK, HB * WP)
            .transpose(0, 1, 3, 2, 4)                      # n, it, k, p, bc
            .astype(bf16))
    dx = np.ascontiguousarray(dx)
    # weight transform along kh: [ci_p, (ot, it, k, kw, co_p)]
    g = np.einsum('kj,ocjv->ockv', _G, weight.astype(np.float64))
    g = (g.reshape(CO_T, P, CI_T, P, NK, 3)    # (ot, co_p, it, ci_p, k, kw)
          .transpose(3, 0, 2, 4, 5, 1)         # (ci_p, ot, it, k, kw, co_p)
          .reshape(P, CO_T * CI_T * NK * 3 * P))
    wT = np.ascontiguousarray(g).astype(bf16)
    bT = np.ascontiguousarray(np.asarray(bias, np.float32).reshape(CO_T, P).T)
    return dx, wT, bT


def kernel(ip, weight, bias, _trace=False, _trace_kwargs=None):
    ip = np.asarray(ip, dtype=np.float32)
    weight = np.asarray(weight, dtype=np.float32)
    bias = np.asarray(bias, dtype=np.float32)

    if "nc" not in _cached:
        _cached["nc"] = _build_nc()
    nc = _cached["nc"]

    dx, wT, bT = _prep_inputs(ip, weight, bias)
    in_maps = [
        {"dx": dx[i * NIMG:(i + 1) * NIMG], "weight": wT, "bias": bT}
        for i in range(N_CORES)
    ]
    res = run_bass_kernel_spmd(
        nc, in_maps, core_ids=list(range(N_CORES)),
        trace=_trace, **(_trace_kwargs or {}),
    )
    out = np.concatenate([r["out"] for r in res.results], axis=0)
    if _trace:
        return out, res
    return out
